# Initial kernel scaffold
#
"""Multi-head attention TRN2 kernel (B=2, N=2048, D=1024, H=16).

Sharding: tensor-parallel over heads. Each of the 8 cores owns 2 heads
(both batch elements) end-to-end through QKV projection and attention,
then the per-head attention outputs are AllGathered (per batch element)
and each core computes a 128-column slice of the output projection.

All matmuls run in float32r (TF32-like fp32 @ full PE rate). Softmax is
computed without max-subtraction (scores are O(5) for this distribution,
exp is safe in fp32) as exp(S^T * scale) directly in the key-major
(transposed) layout, with the row-sum obtained by augmenting V with a
ones column, so no cross-partition reductions are ever needed.

Self-contained: hardcodes shapes from the problem spec.
"""

import sys

for _p in ("/opt/trn_rl_repo", "/root/.axon_site/_ro/trn_rl_repo"):
    if _p not in sys.path:
        sys.path.append(_p)

import numpy as np
from contextlib import ExitStack

import concourse.bass as bass
import concourse.tile as tile
from concourse import mybir, bacc
from concourse.bass_utils import run_bass_kernel_spmd

F32 = mybir.dt.float32
F32R = mybir.dt.float32r
EXP = mybir.ActivationFunctionType.Exp

B = 2
N = 2048
D = 1024
H = 16
DEPTH = 64
TOK = B * N            # 4096 tokens total (both batches)
KC = D // 128          # 8 contraction chunks of 128
NBLK = TOK // 512      # 8 token blocks for streaming projections
SCALE = 1.0 / np.sqrt(DEPTH)
NCORES = 8
IBLK = 1024            # query-block width in attention
NSUB = IBLK // 512     # matmuls per psum tile (N<=512 for 4-byte dtypes)


def build_nc():
    """Build the per-core kernel. SPMD: all cores run this program on their
    own input slices."""
    nc = bacc.Bacc(None)

    xt = nc.dram_tensor("xt", [D, TOK], F32, kind="ExternalInput")
    wq = nc.dram_tensor("wq", [D, 128], F32, kind="ExternalInput")
    wk = nc.dram_tensor("wk", [D, 128], F32, kind="ExternalInput")
    wv = nc.dram_tensor("wv", [D, 128], F32, kind="ExternalInput")
    wp = nc.dram_tensor("wp", [D, 128], F32, kind="ExternalInput")
    bq = nc.dram_tensor("bq", [128, 1], F32, kind="ExternalInput")
    bk = nc.dram_tensor("bk", [128, 1], F32, kind="ExternalInput")
    bv = nc.dram_tensor("bv", [128, 1], F32, kind="ExternalInput")
    bp = nc.dram_tensor("bp", [128, 1], F32, kind="ExternalInput")
    ident = nc.dram_tensor("ident", [128, 128], F32, kind="ExternalInput")
    out = nc.dram_tensor("o", [128, TOK], F32, kind="ExternalOutput")

    # Collective staging (per batch element so the b=0 AllGather overlaps
    # b=1 attention).
    ag_in = [nc.dram_tensor(f"ag_in{b}", [128, N], F32R) for b in range(B)]
    ag_out = [
        nc.dram_tensor(f"ag_out{b}", [D, N], F32R, addr_space="Shared")
        for b in range(B)
    ]

    xt_r = xt.rearrange("(kc p) t -> p kc t", p=128)

    with tile.TileContext(nc) as tc, ExitStack() as ctx:
        wpool = ctx.enter_context(tc.tile_pool(name="w", bufs=1))
        qkpool = ctx.enter_context(tc.tile_pool(name="qk", bufs=1))
        vpool = ctx.enter_context(tc.tile_pool(name="v2", bufs=1))
        opool = ctx.enter_context(tc.tile_pool(name="oh", bufs=1))

        # ---- weights / constants ----
        w_q = wpool.tile([128, KC, 128], F32R, tag="w_q")
        w_k = wpool.tile([128, KC, 128], F32R, tag="w_k")
        w_v = wpool.tile([128, KC, 128], F32R, tag="w_v")
        w_p = wpool.tile([128, KC, 128], F32R, tag="w_p")
        for t, src in ((w_q, wq), (w_k, wk), (w_v, wv), (w_p, wp)):
            nc.sync.dma_start(
                out=t, in_=src.rearrange("(kc p) m -> p kc m", p=128).bitcast(F32R)
            )
        b_q = wpool.tile([128, 1], F32, tag="b_q")
        b_k = wpool.tile([128, 1], F32, tag="b_k")
        b_v = wpool.tile([128, 1], F32, tag="b_v")
        b_p = wpool.tile([128, 1], F32, tag="b_p")
        for t, src in ((b_q, bq), (b_k, bk), (b_v, bv), (b_p, bp)):
            nc.sync.dma_start(out=t, in_=src[:])
        id_t = wpool.tile([128, 128], F32R, tag="id_t")
        nc.sync.dma_start(out=id_t, in_=ident[:].bitcast(F32R))

        # ---- outputs of phase A ----
        # qT/kT: [feature 128 (=2 heads x 64), token 4096]; head hl at rows
        # hl*64:(hl+1)*64 so both S^T operands share a partition base.
        qT = qkpool.tile([128, TOK], F32R, tag="qT")
        kT = qkpool.tile([128, TOK], F32R, tag="kT")
        # V2: [token part, 32 token-chunks, 130]: v_h0 | ones | v_h1 | ones
        V2 = vpool.tile([128, TOK // 128, 130], F32R, tag="V2")
        nc.vector.memset(V2[:, :, 64:65], 1.0)
        nc.vector.memset(V2[:, :, 129:130], 1.0)

        # per-head attention output A^T rows (64 each), full token range
        o_h = [opool.tile([64, TOK], F32R, tag=f"o_h{hl}") for hl in range(2)]

        # ---- phase A: QKV projections (stream x^T in token blocks) ----
        with ExitStack() as actx:
            xpool = actx.enter_context(tc.tile_pool(name="x", bufs=3))
            vtpool = actx.enter_context(tc.tile_pool(name="vt", bufs=2))
            psA = actx.enter_context(tc.tile_pool(name="psA", bufs=2, space="PSUM"))

            for blk in range(NBLK):
                t0 = blk * 512
                xb = xpool.tile([128, KC, 512], F32R, tag="xb")
                nc.sync.dma_start(
                    out=xb, in_=xt_r[:, :, t0 : t0 + 512].bitcast(F32R)
                )
                for name, w_t, b_t in (
                    ("q", w_q, b_q), ("k", w_k, b_k), ("v", w_v, b_v)
                ):
                    ps = psA.tile([128, 512], F32, tag=f"ps_{name}")
                    for kc in range(KC):
                        nc.tensor.matmul(
                            out=ps,
                            lhsT=w_t[:, kc, :],
                            rhs=xb[:, kc, :],
                            start=(kc == 0),
                            stop=(kc == KC - 1),
                        )
                    if name == "q":
                        nc.vector.tensor_scalar_add(
                            out=qT[:, t0 : t0 + 512], in0=ps, scalar1=b_t
                        )
                    elif name == "k":
                        nc.vector.tensor_scalar_add(
                            out=kT[:, t0 : t0 + 512], in0=ps, scalar1=b_t
                        )
                    else:
                        vtmp = vtpool.tile([128, 512], F32R, tag="vtmp")
                        nc.vector.tensor_scalar_add(
                            out=vtmp, in0=ps, scalar1=b_t
                        )
                        # transpose 4x [128,128] -> V2 token chunks
                        for s in range(4):
                            ch = blk * 4 + s
                            ps_t = psA.tile([128, 128], F32, tag="ps_t")
                            nc.tensor.transpose(
                                out=ps_t,
                                in_=vtmp[:, s * 128 : (s + 1) * 128],
                                identity=id_t,
                            )
                            nc.vector.tensor_copy(
                                out=V2[:, ch, 0:64], in_=ps_t[:, 0:64]
                            )
                            nc.vector.tensor_copy(
                                out=V2[:, ch, 65:129], in_=ps_t[:, 64:128]
                            )

        # ---- phase B: attention per (batch, head) + phase C collectives ----
        with ExitStack() as bctx:
            ptpool = bctx.enter_context(tc.tile_pool(name="pt", bufs=3))
            rpool = bctx.enter_context(tc.tile_pool(name="r", bufs=2))
            rdpool = bctx.enter_context(
                tc.tile_pool(name="rd", bufs=2, space="DRAM")
            )
            psS = bctx.enter_context(tc.tile_pool(name="psS", bufs=2, space="PSUM"))
            psO = bctx.enter_context(tc.tile_pool(name="psO", bufs=2, space="PSUM"))

            for b in range(B):
                for hl in range(2):
                    hs = hl * 64
                    voff = hl * 65
                    for ib in range(N // IBLK):
                        i0 = b * N + ib * IBLK
                        ps_o = psO.tile([65, IBLK], F32, tag="ps_o")
                        for jc in range(N // 128):
                            j0 = b * N + jc * 128
                            ps_s = psS.tile([128, IBLK], F32, tag="ps_s")
                            for su in range(NSUB):
                                nc.tensor.matmul(
                                    out=ps_s[:, su * 512 : (su + 1) * 512],
                                    lhsT=kT[hs : hs + 64, j0 : j0 + 128],
                                    rhs=qT[
                                        hs : hs + 64,
                                        i0 + su * 512 : i0 + (su + 1) * 512,
                                    ],
                                    start=True,
                                    stop=True,
                                )
                            pt = ptpool.tile([128, IBLK], F32R, tag="pt")
                            nc.scalar.activation(
                                out=pt, in_=ps_s, func=EXP, scale=float(SCALE)
                            )
                            for su in range(NSUB):
                                nc.tensor.matmul(
                                    out=ps_o[:, su * 512 : (su + 1) * 512],
                                    lhsT=V2[:, (j0 // 128), voff : voff + 65],
                                    rhs=pt[:, su * 512 : (su + 1) * 512],
                                    start=(jc == 0),
                                    stop=(jc == N // 128 - 1),
                                )
                        # normalize: r = 1/rowsum (psum row 64), broadcast
                        # to 64 partitions via a DRAM round-trip
                        rb = rpool.tile([128, IBLK], F32, tag="rb")
                        nc.vector.reciprocal(
                            out=rb[64:65, :], in_=ps_o[64:65, :]
                        )
                        rd = rdpool.tile([1, IBLK], F32, tag="rd")
                        nc.sync.dma_start(out=rd, in_=rb[64:65, :])
                        rr = rpool.tile([64, IBLK], F32, tag="rr")
                        nc.sync.dma_start(
                            out=rr, in_=rd.to_broadcast((64, IBLK))
                        )
                        nc.vector.tensor_mul(
                            out=o_h[hl][:, i0 : i0 + IBLK],
                            in0=ps_o[0:64, :],
                            in1=rr,
                        )
                    # stage this (b, head) block for the AllGather
                    nc.sync.dma_start(
                        out=ag_in[b][hs : hs + 64, :],
                        in_=o_h[hl][:, b * N : (b + 1) * N],
                    )
                nc.gpsimd.collective_compute(
                    "AllGather",
                    mybir.AluOpType.bypass,
                    ins=[ag_in[b][:]],
                    outs=[ag_out[b][:]],
                    replica_groups=[list(range(NCORES))],
                )

        # ---- phase D: output projection (this core's 128 columns) ----
        with ExitStack() as dctx:
            apool = dctx.enter_context(tc.tile_pool(name="ap", bufs=3))
            oupool = dctx.enter_context(tc.tile_pool(name="ou", bufs=2))
            psP = dctx.enter_context(tc.tile_pool(name="psP", bufs=2, space="PSUM"))

            for b in range(B):
                ag_r = ag_out[b].rearrange("(kc p) t -> p kc t", p=128)
                for ib in range(N // 512):
                    i0 = ib * 512
                    ab = apool.tile([128, KC, 512], F32R, tag="ab")
                    nc.sync.dma_start(out=ab, in_=ag_r[:, :, i0 : i0 + 512])
                    ps = psP.tile([128, 512], F32, tag="ps_p")
                    for kc in range(KC):
                        nc.tensor.matmul(
                            out=ps,
                            lhsT=w_p[:, kc, :],
                            rhs=ab[:, kc, :],
                            start=(kc == 0),
                            stop=(kc == KC - 1),
                        )
                    ot = oupool.tile([128, 512], F32, tag="ot")
                    nc.vector.tensor_scalar_add(out=ot, in0=ps, scalar1=b_p)
                    nc.sync.dma_start(
                        out=out[:, b * N + i0 : b * N + i0 + 512], in_=ot
                    )

    nc.compile()
    return nc


def prep_in_maps(x, Wqkv, bqkv, Wproj, bproj):
    x = np.asarray(x, dtype=np.float32)
    Wqkv = np.asarray(Wqkv, dtype=np.float32)
    bqkv = np.asarray(bqkv, dtype=np.float32)
    Wproj = np.asarray(Wproj, dtype=np.float32)
    bproj = np.asarray(bproj, dtype=np.float32)

    xtn = np.ascontiguousarray(x.reshape(TOK, D).T)  # [D, TOK]
    identity = np.eye(128, dtype=np.float32)

    # qkv column index for (head h, depth d, which): h*192 + d*3 + which
    d_idx = np.arange(DEPTH)
    in_maps = []
    for c in range(NCORES):
        h0, h1 = 2 * c, 2 * c + 1
        qcols = np.concatenate([h0 * 192 + 3 * d_idx, h1 * 192 + 3 * d_idx])
        kcols = qcols + 1
        vcols = qcols + 2
        in_maps.append(
            {
                "xt": xtn,
                "wq": np.ascontiguousarray(Wqkv[:, qcols]),
                "wk": np.ascontiguousarray(Wqkv[:, kcols]),
                "wv": np.ascontiguousarray(Wqkv[:, vcols]),
                "wp": np.ascontiguousarray(Wproj[:, 128 * c : 128 * (c + 1)]),
                "bq": np.ascontiguousarray(bqkv[qcols]).reshape(128, 1),
                "bk": np.ascontiguousarray(bqkv[kcols]).reshape(128, 1),
                "bv": np.ascontiguousarray(bqkv[vcols]).reshape(128, 1),
                "bp": np.ascontiguousarray(
                    bproj[128 * c : 128 * (c + 1)]
                ).reshape(128, 1),
                "ident": identity,
            }
        )
    return in_maps


def assemble(results):
    outT = np.concatenate([r["o"] for r in results], axis=0)  # [D, TOK]
    return np.ascontiguousarray(outT.T).reshape(B, N, D).astype(np.float32)


_NC_CACHE = {}


def get_nc():
    if "nc" not in _NC_CACHE:
        _NC_CACHE["nc"] = build_nc()
    return _NC_CACHE["nc"]


def kernel(x, Wqkv, bqkv, Wproj, bproj):
    nc = get_nc()
    in_maps = prep_in_maps(x, Wqkv, bqkv, Wproj, bproj)
    res = run_bass_kernel_spmd(nc, in_maps, list(range(NCORES)))
    return assemble(res.results)


# revision 14
# speedup vs baseline: 1.3065x; 1.3065x over previous
"""Multi-head attention TRN2 kernel (B=2, N=2048, D=1024, H=16).

Sharding: tensor-parallel over heads. Each of the 8 cores owns 2 heads
(both batch elements) end-to-end through QKV projection and attention,
then the per-head attention outputs are AllGathered (per batch element)
and each core computes a 128-column slice of the output projection.

All matmuls run in float32r (TF32-like fp32 @ full PE rate, ~1.5e-4
scaled error). Softmax runs without max-subtraction (scores are O(5)
here; exp is safe in fp32): S^T is computed directly in key-major
layout via matmul(lhsT=kT, rhs=qT), exp'd elementwise on ScalarE, and
the softmax denominator comes from a ones-column appended to V in the
P^T@V matmul — no cross-partition reductions anywhere.

All pools are flat (top-level) with PSUM banks time-multiplexed by tag,
so the scheduler can interleave QKV, attention, collectives, and the
output projection across engines.

Self-contained: hardcodes shapes from the problem spec.
"""

import sys

for _p in ("/opt/trn_rl_repo", "/root/.axon_site/_ro/trn_rl_repo"):
    if _p not in sys.path:
        sys.path.append(_p)

import numpy as np
from contextlib import ExitStack

import concourse.bass as bass
import concourse.tile as tile
from concourse import mybir, bacc
from concourse.bass_utils import run_bass_kernel_spmd

F32 = mybir.dt.float32
F32R = mybir.dt.float32r
BF16 = mybir.dt.bfloat16
EXP = mybir.ActivationFunctionType.Exp

B = 2
N = 2048
D = 1024
H = 16
DEPTH = 64
TOK = B * N            # 4096 tokens total (both batches)
KC = D // 128          # 8 contraction chunks of 128
NBLK = TOK // 512      # 8 token blocks for streaming projections
SCALE = 1.0 / np.sqrt(DEPTH)
NCORES = 8
IBLK = 1024            # query-block width in attention
NSUB = IBLK // 512     # matmuls per psum tile (N<=512 for 4-byte dtypes)


def build_nc(reps=1, with_collective=True, qkv_dt=F32R, attn_dt=F32R,
             proj_dt=F32R):
    """Build the per-core kernel program.

    reps>1 wraps the compute in a For_i hardware loop for benchmarking
    (collectives are skipped: they cannot appear inside control flow).

    qkv_dt: dtype of x^T and QKV weights (the QKV matmuls).
    attn_dt: dtype of q^T/k^T/V2/P^T (the S^T and P^T@V matmuls).
    proj_dt: dtype of the AllGathered A^T and Wproj (projection matmuls).
    f32r ~1.5e-4 scaled err @2cyc/row; bf16 ~2e-3 @1cyc/row.
    """
    bench = reps > 1
    nc = bacc.Bacc(None)

    def dram_dt(dt):
        return F32 if dt == F32R else dt

    def cast(ap, dt):
        return ap.bitcast(F32R) if dt == F32R else ap

    xt = nc.dram_tensor("xt", [D, TOK], dram_dt(qkv_dt), kind="ExternalInput")
    wq = nc.dram_tensor("wq", [D, 128], dram_dt(qkv_dt), kind="ExternalInput")
    wk = nc.dram_tensor("wk", [D, 128], dram_dt(qkv_dt), kind="ExternalInput")
    wv = nc.dram_tensor("wv", [D, 128], dram_dt(qkv_dt), kind="ExternalInput")
    wp = nc.dram_tensor("wp", [D, 128], dram_dt(proj_dt), kind="ExternalInput")
    bq = nc.dram_tensor("bq", [128, 1], F32, kind="ExternalInput")
    bk = nc.dram_tensor("bk", [128, 1], F32, kind="ExternalInput")
    bv = nc.dram_tensor("bv", [128, 1], F32, kind="ExternalInput")
    bp = nc.dram_tensor("bp", [128, 1], F32, kind="ExternalInput")
    ident = nc.dram_tensor(
        "ident", [128, 128], dram_dt(attn_dt), kind="ExternalInput"
    )
    ones = nc.dram_tensor("ones", [128, 1], dram_dt(attn_dt), kind="ExternalInput")
    out = nc.dram_tensor("o", [128, TOK], F32, kind="ExternalOutput")

    # Collective staging (split per batch element so the b=0 AllGather
    # overlaps b=1 attention).
    HALF = N // 2
    ag_in = [
        [nc.dram_tensor(f"ag_in{b}_{h}", [128, HALF], proj_dt) for h in range(2)]
        for b in range(B)
    ]
    ag_out = [
        [
            nc.dram_tensor(f"ag_out{b}_{h}", [D, HALF], proj_dt,
                           addr_space="Shared")
            for h in range(2)
        ]
        for b in range(B)
    ]

    xt_r = xt.rearrange("(kc p) t -> p kc t", p=128)

    with tile.TileContext(nc) as tc, ExitStack() as ctx:
        wpool = ctx.enter_context(tc.tile_pool(name="w", bufs=1))
        qkpool = ctx.enter_context(tc.tile_pool(name="qk", bufs=1))
        vpool = ctx.enter_context(tc.tile_pool(name="v2", bufs=1))
        xpool = ctx.enter_context(tc.tile_pool(name="x", bufs=3))
        vtpool = ctx.enter_context(tc.tile_pool(name="vt", bufs=2))
        ptpool = ctx.enter_context(tc.tile_pool(name="pt", bufs=3))
        unpool = ctx.enter_context(tc.tile_pool(name="un", bufs=3))
        rpool = ctx.enter_context(tc.tile_pool(name="r", bufs=1))
        rdpool = ctx.enter_context(tc.tile_pool(name="rd", bufs=2, space="DRAM"))
        apool = ctx.enter_context(tc.tile_pool(name="ap", bufs=2))
        oupool = ctx.enter_context(tc.tile_pool(name="ou", bufs=2))
        # (SBUF/partition budget: w 17 + qk 32 + v2 16.6 + x 48 + vt 4 +
        #  pt 12 + un 24 + r 16 + ap 32 + ou 4 ~= 205 KB of 208)
        # PSUM budget (8 banks of 2KB/partition):
        #   ps_one (bufs=1): po [65,1024] (2x1)                   -> 2 banks
        #   ps_mm (bufs=2): mmA [128,512] (1x2) shared by qkv-proj groups,
        #                   v-transposes and out-proj groups      -> 2 banks
        #   ps_two (bufs=2): ss [128,1024] (2x2)                  -> 4 banks
        ps_one = ctx.enter_context(tc.tile_pool(name="ps1", bufs=1, space="PSUM"))
        ps_mm = ctx.enter_context(tc.tile_pool(name="psm", bufs=2, space="PSUM"))
        ps_two = ctx.enter_context(tc.tile_pool(name="ps2", bufs=2, space="PSUM"))

        # ---- weights / constants (loaded once, outside any rep loop) ----
        w_q = wpool.tile([128, KC, 128], qkv_dt, tag="w_q")
        w_k = wpool.tile([128, KC, 128], qkv_dt, tag="w_k")
        w_v = wpool.tile([128, KC, 128], qkv_dt, tag="w_v")
        w_p = wpool.tile([128, KC, 128], proj_dt, tag="w_p")
        for t, src, dt in (
            (w_q, wq, qkv_dt), (w_k, wk, qkv_dt), (w_v, wv, qkv_dt),
            (w_p, wp, proj_dt),
        ):
            nc.sync.dma_start(
                out=t, in_=cast(src.rearrange("(kc p) m -> p kc m", p=128), dt)
            )
        b_q = wpool.tile([128, 1], F32, tag="b_q")
        b_k = wpool.tile([128, 1], F32, tag="b_k")
        b_v = wpool.tile([128, 1], F32, tag="b_v")
        b_p = wpool.tile([128, 1], F32, tag="b_p")
        for t, src in ((b_q, bq), (b_k, bk), (b_v, bv), (b_p, bp)):
            nc.gpsimd.dma_start(out=t, in_=src[:])
        id_t = wpool.tile([128, 128], attn_dt, tag="id_t")
        nc.gpsimd.dma_start(out=id_t, in_=cast(ident[:], attn_dt))

        # qT/kT: [feature 128 (= 2 heads x 64), token 4096]; head hl in rows
        # hl*64:(hl+1)*64 so both S^T operands share a partition base.
        qT = qkpool.tile([128, TOK], attn_dt, tag="qT")
        kT = qkpool.tile([128, TOK], attn_dt, tag="kT")
        # V2: [token part, 32 token-chunks, 130]: v_h0 | ones | v_h1 | ones
        V2 = vpool.tile([128, TOK // 128, 130], attn_dt, tag="V2")
        nc.gpsimd.dma_start(
            out=V2[:, :, 64:65],
            in_=cast(ones[:].to_broadcast((128, TOK // 128, 1)), attn_dt),
        )
        nc.gpsimd.dma_start(
            out=V2[:, :, 129:130],
            in_=cast(ones[:].to_broadcast((128, TOK // 128, 1)), attn_dt),
        )

        rep_ctx = ExitStack()
        if bench:
            rep_ctx.enter_context(
                tc.For_i(
                    0,
                    reps,
                    1,
                    hint_engines=(
                        mybir.EngineType.PE,
                        mybir.EngineType.Activation,
                        mybir.EngineType.DVE,
                        mybir.EngineType.SP,
                    ),
                )
            )

        # ================= phase A: QKV projections =================
        for blk in range(NBLK):
            t0 = blk * 512
            xb = xpool.tile([128, KC, 512], qkv_dt, tag="xb")
            eng = nc.sync if blk % 2 == 0 else nc.gpsimd
            eng.dma_start(out=xb, in_=cast(xt_r[:, :, t0 : t0 + 512], qkv_dt))
            for name, w_t, b_t in (
                ("q", w_q, b_q), ("k", w_k, b_k), ("v", w_v, b_v)
            ):
                ps = ps_mm.tile([128, 512], F32, tag="mmA")
                for kc in range(KC):
                    nc.tensor.matmul(
                        out=ps,
                        lhsT=w_t[:, kc, :],
                        rhs=xb[:, kc, :],
                        start=(kc == 0),
                        stop=(kc == KC - 1),
                    )
                if name == "q":
                    nc.vector.tensor_scalar_add(
                        out=qT[:, t0 : t0 + 512], in0=ps, scalar1=b_t
                    )
                elif name == "k":
                    nc.vector.tensor_scalar_add(
                        out=kT[:, t0 : t0 + 512], in0=ps, scalar1=b_t
                    )
                else:
                    vtmp = vtpool.tile([128, 512], attn_dt, tag="vtmp")
                    nc.vector.tensor_scalar_add(out=vtmp, in0=ps, scalar1=b_t)
                    # transpose 4x [128,128] -> V2 token chunks
                    for s in range(4):
                        ch = blk * 4 + s
                        ps_t = ps_mm.tile([128, 128], attn_dt, tag="mmA")
                        nc.tensor.transpose(
                            out=ps_t,
                            in_=vtmp[:, s * 128 : (s + 1) * 128],
                            identity=id_t,
                        )
                        nc.vector.tensor_copy(out=V2[:, ch, 0:64], in_=ps_t[:, 0:64])
                        nc.vector.tensor_copy(
                            out=V2[:, ch, 65:129], in_=ps_t[:, 64:128]
                        )

        # ===== phase B: attention per (batch, head) + AllGathers =====
        for b in range(B):
            for hl in range(2):
                hs = hl * 64
                voff = hl * 65
                for ib in range(N // IBLK):
                    i0 = b * N + ib * IBLK
                    il = ib * IBLK  # local (within-batch) offset
                    ps_o = ps_one.tile([65, IBLK], F32, tag="po")
                    NJC = N // 128

                    def s_step(jc):
                        j0 = b * N + jc * 128
                        ps_s = ps_two.tile([128, IBLK], F32, tag="ss")
                        for su in range(NSUB):
                            nc.tensor.matmul(
                                out=ps_s[:, su * 512 : (su + 1) * 512],
                                lhsT=kT[hs : hs + 64, j0 : j0 + 128],
                                rhs=qT[
                                    hs : hs + 64,
                                    i0 + su * 512 : i0 + (su + 1) * 512,
                                ],
                                start=True,
                                stop=True,
                            )
                        pt = ptpool.tile([128, IBLK], attn_dt, tag="pt")
                        nc.scalar.activation(
                            out=pt, in_=ps_s, func=EXP, scale=float(SCALE)
                        )
                        return pt

                    # software pipeline: keep S^T one step ahead of PV in
                    # the PE stream so PE never waits on ScalarE's exp
                    pts = s_step(0)
                    for jc in range(NJC):
                        pt_cur = pts
                        if jc + 1 < NJC:
                            pts = s_step(jc + 1)
                        for su in range(NSUB):
                            nc.tensor.matmul(
                                out=ps_o[:, su * 512 : (su + 1) * 512],
                                lhsT=V2[:, ((b * N + jc * 128) // 128), voff : voff + 65],
                                rhs=pt_cur[:, su * 512 : (su + 1) * 512],
                                start=(jc == 0),
                                stop=(jc == NJC - 1),
                            )
                    # early-evict from PSUM: unnormalized rows + reciprocal,
                    # then normalize via a DRAM broadcast round-trip and
                    # stream the chunk straight to AllGather staging
                    un = unpool.tile([64, IBLK], F32, tag="un")
                    nc.vector.tensor_copy(out=un, in_=ps_o[0:64, :])
                    rb = rpool.tile([128, IBLK], F32, tag="rb")
                    nc.vector.reciprocal(out=rb[64:65, :], in_=ps_o[64:65, :])
                    rd = rdpool.tile([1, IBLK], F32, tag="rd")
                    nc.gpsimd.dma_start(out=rd, in_=rb[64:65, :])
                    rr = rpool.tile([64, IBLK], F32, tag="rr")
                    nc.gpsimd.dma_start(out=rr, in_=rd.to_broadcast((64, IBLK)))
                    unr = unpool.tile([64, IBLK], proj_dt, tag="unr")
                    nc.vector.tensor_mul(out=unr, in0=un, in1=rr)
                    nc.gpsimd.dma_start(
                        out=ag_in[b][ib][hs : hs + 64, :], in_=unr
                    )
                if with_collective and hl == 1:
                    # both heads of (b, half) staged: gather each half
                    for hf in range(2):
                        nc.gpsimd.collective_compute(
                            "AllGather",
                            mybir.AluOpType.bypass,
                            ins=[ag_in[b][hf][:]],
                            outs=[ag_out[b][hf][:]],
                            replica_groups=[list(range(NCORES))],
                        )

        # ======= phase D: output projection (128 columns/core) =======
        for b in range(B):
            for ib in range(N // 512):
                hf = ib // 2
                ag_r = ag_out[b][hf].rearrange("(kc p) t -> p kc t", p=128)
                i0 = (ib % 2) * 512
                ab = apool.tile([128, KC, 512], proj_dt, tag="ab")
                eng = nc.sync if ib % 2 == 0 else nc.gpsimd
                eng.dma_start(out=ab, in_=ag_r[:, :, i0 : i0 + 512])
                ps = ps_mm.tile([128, 512], F32, tag="mmA")
                for kc in range(KC):
                    nc.tensor.matmul(
                        out=ps,
                        lhsT=w_p[:, kc, :],
                        rhs=ab[:, kc, :],
                        start=(kc == 0),
                        stop=(kc == KC - 1),
                    )
                ot = oupool.tile([128, 512], F32, tag="ot")
                nc.vector.tensor_scalar_add(out=ot, in0=ps, scalar1=b_p)
                nc.sync.dma_start(
                    out=out[:, b * N + ib * 512 : b * N + (ib + 1) * 512],
                    in_=ot,
                )

        rep_ctx.close()

    nc.compile()
    return nc


def np_dt(dt):
    return mybir.dt.np(F32 if dt == F32R else dt)


def prep_in_maps(x, Wqkv, bqkv, Wproj, bproj, qkv_dt=F32R, attn_dt=F32R,
                 proj_dt=F32R):
    x = np.asarray(x, dtype=np.float32)
    Wqkv = np.asarray(Wqkv, dtype=np.float32)
    bqkv = np.asarray(bqkv, dtype=np.float32)
    Wproj = np.asarray(Wproj, dtype=np.float32)
    bproj = np.asarray(bproj, dtype=np.float32)

    xtn = np.ascontiguousarray(x.reshape(TOK, D).T).astype(np_dt(qkv_dt))
    identity = np.eye(128, dtype=np_dt(attn_dt))
    ones_col = np.ones((128, 1), dtype=np_dt(attn_dt))

    # AllGather output rows are rank-major: row c*128 + hl*64 + d holds
    # feature (2c+hl)*64 + d; permute Wproj's contraction rows to match.
    wp_row_perm = np.empty(D, dtype=np.int64)
    for cc in range(NCORES):
        for hlhl in range(2):
            rows = np.arange(64)
            wp_row_perm[cc * 128 + hlhl * 64 + rows] = (2 * cc + hlhl) * 64 + rows

    # qkv column index for (head h, depth d, which): h*192 + d*3 + which
    d_idx = np.arange(DEPTH)
    in_maps = []
    for c in range(NCORES):
        h0, h1 = 2 * c, 2 * c + 1
        qcols = np.concatenate([h0 * 192 + 3 * d_idx, h1 * 192 + 3 * d_idx])
        kcols = qcols + 1
        vcols = qcols + 2
        in_maps.append(
            {
                "xt": xtn,
                "wq": np.ascontiguousarray(Wqkv[:, qcols]).astype(np_dt(qkv_dt)),
                "wk": np.ascontiguousarray(Wqkv[:, kcols]).astype(np_dt(qkv_dt)),
                "wv": np.ascontiguousarray(Wqkv[:, vcols]).astype(np_dt(qkv_dt)),
                "wp": np.ascontiguousarray(
                    Wproj[wp_row_perm, 128 * c : 128 * (c + 1)]
                ).astype(np_dt(proj_dt)),
                "bq": np.ascontiguousarray(bqkv[qcols]).reshape(128, 1),
                "bk": np.ascontiguousarray(bqkv[kcols]).reshape(128, 1),
                "bv": np.ascontiguousarray(bqkv[vcols]).reshape(128, 1),
                "bp": np.ascontiguousarray(
                    bproj[128 * c : 128 * (c + 1)]
                ).reshape(128, 1),
                "ident": identity,
                "ones": ones_col,
            }
        )
    return in_maps


def assemble(results):
    outT = np.concatenate([r["o"] for r in results], axis=0)  # [D, TOK]
    return np.ascontiguousarray(outT.T).reshape(B, N, D).astype(np.float32)


# Chosen dtype config (see build_nc docstring for the tradeoff).
CONFIG = {"qkv_dt": F32R, "attn_dt": F32R, "proj_dt": F32R}

_NC_CACHE = {}


def get_nc():
    if "nc" not in _NC_CACHE:
        _NC_CACHE["nc"] = build_nc(**CONFIG)
    return _NC_CACHE["nc"]


def kernel(x, Wqkv, bqkv, Wproj, bproj):
    nc = get_nc()
    in_maps = prep_in_maps(x, Wqkv, bqkv, Wproj, bproj, **CONFIG)
    res = run_bass_kernel_spmd(nc, in_maps, list(range(NCORES)))
    return assemble(res.results)
